# revision 13
# baseline (speedup 1.0000x reference)
"""Trainium2 Bass kernel for a dense transformer block — head-sharded variant.

Sharding: core c in 0..7 owns HEADS 2c, 2c+1 for BOTH batches. It computes
Q/K/V projections (1/8 of the work, no duplication) and causal attention for
its two heads over both 2048-token sequences. The normalized attention
outputs ([128 ch, 512] per (batch, window)) are exchanged with ONE 8-core
AllToAll so that core c = (b, q) ends up with all 1024 attention channels for
its contiguous token quarter q of batch b. It then does residual + LN1-folded
MLP + LN2 on those 512 tokens (identical to the improved zero-comm kernel).

This removes the 4x duplicated K/V projection work of the residue-sharded
variant (~93us of PE time per core) at the cost of a ~25us AllToAll.
"""

import contextlib
import sys
import types

import numpy as np
import ml_dtypes

if "antenv.axon_hooks" not in sys.modules:
    _hook_mod = types.ModuleType("antenv.axon_hooks")
    _hook_state = {"h": None}
    _hook_mod.set_axon_ntff_profile_hook = lambda h: _hook_state.__setitem__("h", h)
    _hook_mod.get_axon_ntff_profile_hook = lambda: _hook_state["h"]
    sys.modules["antenv.axon_hooks"] = _hook_mod
    try:
        import antenv

        antenv.axon_hooks = _hook_mod
    except ImportError:
        pass
    try:
        from trn_agent_boot.trn_boot import _ntff_profile_via_ctypes

        _hook_state["h"] = _ntff_profile_via_ctypes("/opt/axon/libaxon_pjrt.so")
    except Exception:
        pass

import concourse.bass as bass  # noqa: E402
import concourse.mybir as mybir  # noqa: E402
import concourse.tile as tile  # noqa: E402
from concourse import bacc  # noqa: E402
from concourse.bass_utils import run_bass_kernel_spmd  # noqa: E402

# ---- problem constants ------------------------------------------------------
B = 2
L = 2048
DIM = 1024
HEAD = 16
HD = 64
HID = 4 * DIM
EPS = 1e-5
P = 128
NQ = 512             # own tokens per core (contiguous quarter)
CB = DIM // P        # 8
EB = HID // P        # 32
NKB = L // P         # 16
NW = 4               # query windows of 512 per batch
SC = 1.0 / np.sqrt(HD)

F32 = mybir.dt.float32
MM = mybir.dt.bfloat16
NP_MM = ml_dtypes.bfloat16

_CACHE = {}


def _build_program():
    nc = bacc.Bacc("TRN2", target_bir_lowering=False, debug=False,
                   enable_asserts=True, num_devices=8)

    d_xbT = nc.dram_tensor("xbT2", [P, CB, B, L], MM, kind="ExternalInput").ap()
    d_xqf = nc.dram_tensor("xqTf", [P, CB, NQ], F32, kind="ExternalInput").ap()
    d_wq = nc.dram_tensor("Wq", [P, CB, P], MM, kind="ExternalInput").ap()
    d_wk = nc.dram_tensor("Wk", [P, CB, P], MM, kind="ExternalInput").ap()
    d_wv = nc.dram_tensor("Wv", [P, CB, P], MM, kind="ExternalInput").ap()
    d_w1 = nc.dram_tensor("W1", [EB, P, CB, P], MM, kind="ExternalInput").ap()
    d_w1c = nc.dram_tensor("W1c", [EB, P, P], MM, kind="ExternalInput").ap()
    d_bw1 = nc.dram_tensor("bW1", [P, EB], F32, kind="ExternalInput").ap()
    d_w2 = nc.dram_tensor("W2", [P, EB, DIM], MM, kind="ExternalInput").ap()
    d_cos = nc.dram_tensor("cosT", [P, L], F32, kind="ExternalInput").ap()
    d_sin = nc.dram_tensor("sinT", [P, L], F32, kind="ExternalInput").ap()
    d_mask = nc.dram_tensor("mask2", [P, 2, P], MM, kind="ExternalInput").ap()
    d_gam = nc.dram_tensor("gammaT", [P, CB], F32, kind="ExternalInput").ap()
    d_bet = nc.dram_tensor("betaT", [P, CB], F32, kind="ExternalInput").ap()
    d_out = nc.dram_tensor("outT", [DIM, NQ], F32, kind="ExternalOutput").ap()

    AF = mybir.ActivationFunctionType
    OP = mybir.AluOpType

    with tile.TileContext(nc) as tc, contextlib.ExitStack() as ctx:
        small = ctx.enter_context(tc.tile_pool(name="small", bufs=1))
        dram = ctx.enter_context(tc.tile_pool(name="dram", bufs=1, space="DRAM"))

        gam = small.tile([P, CB], F32)
        bet = small.tile([P, CB], F32)
        ones128 = small.tile([P, P], MM)
        epst = small.tile([1, 1], F32)
        mask2 = small.tile([P, 2, P], MM)

        a2a_src = dram.tile([8, P, NQ], MM)
        a2a_dst = dram.tile([8, P, NQ], MM)
        warm_src = dram.tile([8, 64], MM)
        warm_dst = dram.tile([8, 64], MM)

        # ======================= scope 1: QKV + attention ====================
        with tc.tile_pool(name="qkv", bufs=1) as qkv:
            kT = qkv.tile([P, B, L], MM)
            qT = qkv.tile([P, B, L], MM)
            vaug = qkv.tile([P, B, NKB, 2 * (HD + 1)], MM)
            va3 = vaug.rearrange("p b t (h c) -> p b t h c", c=HD + 1)

            # ---------------- phase A: QKV projections + RoPE ----------------
            with (
                tc.tile_pool(name="xin", bufs=1) as xin,
                tc.tile_pool(name="wpool", bufs=1) as wpool,
                tc.tile_pool(name="ropetmp", bufs=2) as ropetmp,
                tc.tile_pool(name="tabs", bufs=1) as tabs,
                tc.tile_pool(name="psA", bufs=4, space="PSUM") as psA,
                tc.tile_pool(name="psV", bufs=4, space="PSUM") as psV,
            ):
                # critical-path DMAs first
                xbT = xin.tile([P, CB, B, L], MM)
                nc.sync.dma_start(xbT[:, :, 0, 0:512], d_xbT[:, :, 0, 0:512])
                wq_t = wpool.tile([P, CB, P], MM)
                nc.sync.dma_start(wq_t, d_wq)
                cosT = tabs.tile([P, L], F32)
                nc.sync.dma_start(cosT, d_cos)
                sinT = tabs.tile([P, L], F32)
                nc.sync.dma_start(sinT, d_sin)
                for b in range(B):
                    for t in range(4):
                        if b == 0 and t == 0:
                            continue
                        nc.sync.dma_start(xbT[:, :, b, t * 512:(t + 1) * 512],
                                          d_xbT[:, :, b, t * 512:(t + 1) * 512])
                wk_t = wpool.tile([P, CB, P], MM)
                nc.sync.dma_start(wk_t, d_wk)
                wv_t = wpool.tile([P, CB, P], MM)
                nc.sync.dma_start(wv_t, d_wv)
                nc.sync.dma_start(gam, d_gam)
                nc.sync.dma_start(bet, d_bet)
                nc.sync.dma_start(mask2, d_mask)
                nc.vector.memset(ones128, 1.0)
                nc.vector.memset(epst, EPS)
                nc.vector.memset(va3[:, :, :, :, HD:HD + 1], 1.0)
                # warm up the collective channel so the real AllToAll at the
                # attention->MLP boundary doesn't pay cold-start setup
                nc.sync.dma_start(warm_src, ones128[0:8, 0:64])
                nc.gpsimd.collective_compute(
                    "AllToAll", mybir.AluOpType.bypass,
                    replica_groups=[[0, 1, 2, 3, 4, 5, 6, 7]],
                    ins=[warm_src.opt()], outs=[warm_dst.opt()])

                def rope_evac(ps, cosS, sinS, out_slice):
                    raw = ropetmp.tile([P, 512], MM, tag="rope_raw")
                    nc.scalar.copy(raw, ps)
                    nc.vector.tensor_mul(out_slice, ps, cosS)
                    swp = ropetmp.tile([P, 512], MM, tag="rope_swp")
                    for g in range(4):
                        s = (g ^ 1) * 32
                        nc.sync.dma_start(swp[g * 32:(g + 1) * 32, :],
                                          raw[s:s + 32, :])
                    tmp = ropetmp.tile([P, 512], MM, tag="rope_tmp")
                    nc.vector.tensor_mul(tmp, swp, sinS)
                    nc.vector.tensor_add(out_slice, out_slice, tmp)

                for wt, dst in ((wq_t, qT), (wk_t, kT)):
                    for b in range(B):
                        for t in range(4):
                            ps_q = psA.tile([P, 512], F32, tag="psA")
                            for kb in range(CB):
                                nc.tensor.matmul(
                                    ps_q, lhsT=wt[:, kb, :],
                                    rhs=xbT[:, kb, b, t * 512:(t + 1) * 512],
                                    start=(kb == 0), stop=(kb == CB - 1))
                            rope_evac(ps_q, cosT[:, t * 512:(t + 1) * 512],
                                      sinT[:, t * 512:(t + 1) * 512],
                                      dst[:, b, t * 512:(t + 1) * 512])

                for b in range(B):
                    for tb in range(NKB):
                        ps_v = psV.tile([P, P], F32, tag="psV")
                        for kb in range(CB):
                            nc.tensor.matmul(
                                ps_v, lhsT=xbT[:, kb, b, tb * P:(tb + 1) * P],
                                rhs=wv_t[:, kb, :],
                                start=(kb == 0), stop=(kb == CB - 1))
                        nc.scalar.copy(
                            va3[:, b, tb, :, 0:HD],
                            ps_v.rearrange("p (h c) -> p h c", c=HD))

            # ---------------- phase B: attention -----------------------------
            with (
                tc.tile_pool(name="attn", bufs=4) as attn,
                tc.tile_pool(name="attv", bufs=2) as attv,
                tc.tile_pool(name="psS", bufs=3, space="PSUM") as psS,
                tc.tile_pool(name="psO", bufs=1, space="PSUM") as psO,
            ):
                for b in range(B):
                    for w in range(NW):
                        nkb_w = 4 * w + 4
                        ps_oA = psO.tile([65, 512], F32, tag="ps_oA")
                        ps_oB = psO.tile([65, 512], F32, tag="ps_oB")
                        ps_s = [None] * nkb_w
                        ex = [None] * nkb_w
                        jos = [max(0, (kb - 4 * w) * P) for kb in range(nkb_w)]

                        def scores(kb, b=b, w=w, ps_s=ps_s, ex=ex, jos=jos):
                            jo = jos[kb]
                            wdt = 512 - jo
                            ps = psS.tile([P, 2, 512], F32, tag="ps_s")
                            nc.tensor.matmul(
                                ps[:, 0, :wdt],
                                lhsT=kT[0:64, b, kb * P:(kb + 1) * P],
                                rhs=qT[0:64, b, w * 512 + jo:(w + 1) * 512],
                                start=True, stop=True)
                            nc.tensor.matmul(
                                ps[:, 1, :wdt],
                                lhsT=kT[64:128, b, kb * P:(kb + 1) * P],
                                rhs=qT[64:128, b, w * 512 + jo:(w + 1) * 512],
                                start=True, stop=True)
                            if kb >= 4 * w:
                                nc.vector.tensor_add(ps[:, :, 0:P],
                                                     ps[:, :, 0:P], mask2)
                            ps_s[kb] = ps
                            e = attn.tile([P, 2, 512], MM, tag="ex")
                            nc.scalar.activation(out=e[:, :, :wdt],
                                                 in_=ps[:, :, :wdt],
                                                 func=AF.Exp, scale=float(SC))
                            ex[kb] = e

                        def av(kb, b=b, w=w, ps_oA=ps_oA, ps_oB=ps_oB,
                               ex=ex, jos=jos, nkb_w=nkb_w):
                            jo = jos[kb]
                            wdt = 512 - jo
                            nc.tensor.matmul(
                                ps_oA[:, jo:], lhsT=va3[:, b, kb, 0, :],
                                rhs=ex[kb][:, 0, :wdt],
                                start=(kb == 0), stop=(kb == nkb_w - 1))
                            nc.tensor.matmul(
                                ps_oB[:, jo:], lhsT=va3[:, b, kb, 1, :],
                                rhs=ex[kb][:, 1, :wdt],
                                start=(kb == 0), stop=(kb == nkb_w - 1))

                        scores(0)
                        if nkb_w > 1:
                            scores(1)
                        for kb in range(nkb_w):
                            if kb + 2 < nkb_w:
                                scores(kb + 2)
                            av(kb)

                        h1w = attv.tile([P, 512], MM, tag="h1w")
                        for hh, ps_o in ((0, ps_oA), (1, ps_oB)):
                            cpy = attv.tile([65, 512], F32, tag=f"ocpy{hh}")
                            nc.vector.tensor_copy(cpy, ps_o)
                            rec = attv.tile([1, 512], F32, tag=f"rec{hh}")
                            nc.vector.reciprocal(rec, cpy[64:65, :])
                            rb = attv.tile([64, 512], F32, tag=f"rb{hh}")
                            nc.gpsimd.partition_broadcast(rb, rec)
                            nc.vector.tensor_mul(h1w[hh * 64:(hh + 1) * 64, :],
                                                 cpy[0:64, :], rb)
                        nc.sync.dma_start(a2a_src[b * NW + w], h1w)

        # ---- the exchange: heads -> token quarters --------------------------
        nc.gpsimd.collective_compute(
            "AllToAll", mybir.AluOpType.bypass,
            replica_groups=[[0, 1, 2, 3, 4, 5, 6, 7]],
            ins=[a2a_src.opt()], outs=[a2a_dst.opt()])

        # ======================= scope 2: MLP with folded LN1, LN2 ===========
        with (
            tc.tile_pool(name="sm2", bufs=1) as stat,
            tc.tile_pool(name="xq2", bufs=1) as xq2,
            tc.tile_pool(name="w2p", bufs=1) as w2p,
            tc.tile_pool(name="hres", bufs=1) as hres,
            tc.tile_pool(name="mlp", bufs=1) as mlp,
            tc.tile_pool(name="evac", bufs=2) as evac,
            tc.tile_pool(name="sqp", bufs=2) as sqp,
            tc.tile_pool(name="w1cs", bufs=2) as w1cs,
            tc.tile_pool(name="w1stream", bufs=2) as w1s,
            tc.tile_pool(name="psC", bufs=1, space="PSUM") as psC,
            tc.tile_pool(name="psD", bufs=2, space="PSUM") as psD,
        ):
            xqf = xq2.tile([P, CB, NQ], F32)
            nc.sync.dma_start(xqf, d_xqf)
            w2 = w2p.tile([P, EB, DIM], MM)
            nc.sync.dma_start(w2, d_w2)
            bw1 = stat.tile([P, EB], F32)
            nc.sync.dma_start(bw1, d_bw1)
            mu_m32 = stat.tile([P, 512], MM)
            nc.vector.memset(mu_m32, 0.0)
            w1_pre = []
            for eb in range(2):
                w1p = w1s.tile([P, CB, P], MM, tag="w1")
                nc.sync.dma_start(w1p, d_w1[eb])
                w1_pre.append(w1p)

            h1m = hres.tile([P, CB, NQ], MM)
            aT = mlp.tile([P, EB, NQ], MM)
            h1nT = mlp.tile([P, CB, NQ], F32)
            h2T = mlp.tile([P, CB, NQ], F32)

            # residual + LN1 stats (pipelined per channel block)
            ps_sum = psC.tile([P, 512], F32, tag="sum")
            ps_sq = psC.tile([P, 512], F32, tag="sq")
            for cb in range(CB):
                hattn = sqp.tile([P, NQ], MM, tag="hat")
                nc.sync.dma_start(hattn, a2a_dst[cb])
                nc.vector.tensor_add(h1m[:, cb, :], hattn, xqf[:, cb, :])
                sq = sqp.tile([P, NQ], MM, tag="sq")
                nc.vector.tensor_mul(sq, h1m[:, cb, :], h1m[:, cb, :])
                nc.tensor.matmul(ps_sum, lhsT=ones128, rhs=h1m[:, cb, :],
                                 start=(cb == 0), stop=(cb == CB - 1))
                nc.tensor.matmul(ps_sq, lhsT=ones128, rhs=sq,
                                 start=(cb == 0), stop=(cb == CB - 1))
            mu = stat.tile([1, 512], F32, tag="mu")
            nc.vector.tensor_scalar_mul(mu, ps_sum[0:1, :], 1.0 / DIM)
            nc.vector.tensor_copy(mu_m32[0:1, :], mu)
            musq = stat.tile([1, 512], F32, tag="musq")
            nc.vector.tensor_mul(musq, mu, mu)
            var = stat.tile([1, 512], F32, tag="var")
            nc.vector.scalar_tensor_tensor(
                out=var, in0=ps_sq[0:1, :], scalar=1.0 / DIM, in1=musq,
                op0=OP.mult, op1=OP.subtract)
            rstd = stat.tile([1, 512], F32, tag="rstd")
            nc.scalar.activation(out=rstd, in_=var, func=AF.Sqrt,
                                 bias=epst[0:1, :], scale=1.0)
            nc.vector.reciprocal(rstd, rstd)
            rstd_b = stat.tile([P, 512], F32, tag="rstd_b")
            nc.gpsimd.partition_broadcast(rstd_b, rstd)
            nmu = stat.tile([1, 512], F32, tag="nmu")
            nc.vector.scalar_tensor_tensor(
                out=nmu, in0=mu, scalar=-1.0, in1=rstd,
                op0=OP.mult, op1=OP.mult)
            nmu_b = stat.tile([P, 512], F32, tag="nmu_b")
            nc.gpsimd.partition_broadcast(nmu_b, nmu)

            # ---- MLP1: aT = gelu(rstd * (W1g^T h1 - colsum ox mu) + betaW1) -
            for eb in range(EB):
                if eb < 2:
                    w1_t = w1_pre[eb]
                else:
                    w1_t = w1s.tile([P, CB, P], MM, tag="w1")
                    nc.sync.dma_start(w1_t, d_w1[eb])
                w1ct = w1cs.tile([P, P], MM, tag="w1c")
                nc.sync.dma_start(w1ct, d_w1c[eb])
                ps_a = psD.tile([P, 512], F32, tag="ps_a")
                for kb in range(CB):
                    nc.tensor.matmul(ps_a, lhsT=w1_t[:, kb, :],
                                     rhs=h1m[:, kb, :],
                                     start=(kb == 0), stop=False)
                nc.tensor.matmul(ps_a, lhsT=w1ct, rhs=mu_m32,
                                 start=False, stop=True)
                zt = evac.tile([P, 512], F32, tag="zt")
                nc.vector.tensor_mul(zt, ps_a, rstd_b)
                nc.scalar.activation(out=aT[:, eb, :], in_=zt, func=AF.Gelu,
                                     bias=bw1[:, eb:eb + 1], scale=1.0)
                if eb % 4 == 3:
                    cb = eb // 4
                    t1 = evac.tile([P, 512], F32, tag="t1")
                    nc.vector.tensor_mul(t1, h1m[:, cb, :], rstd_b)
                    nc.vector.tensor_add(t1, t1, nmu_b)
                    nc.vector.tensor_scalar(
                        out=h1nT[:, cb, :], in0=t1,
                        scalar1=gam[:, cb:cb + 1], scalar2=bet[:, cb:cb + 1],
                        op0=OP.mult, op1=OP.add)

            # ---- MLP2 + residual + pipelined LN2 stats ----------------------
            ps2_sum = psC.tile([P, 512], F32, tag="sum")
            ps2_sq = psC.tile([P, 512], F32, tag="sq")
            for cb in range(CB):
                ps_2 = psD.tile([P, 512], F32, tag="ps_2")
                for eb in range(EB):
                    nc.tensor.matmul(ps_2, lhsT=w2[:, eb, cb * P:(cb + 1) * P],
                                     rhs=aT[:, eb, :],
                                     start=(eb == 0), stop=(eb == EB - 1))
                nc.vector.tensor_add(h2T[:, cb, :], ps_2, h1nT[:, cb, :])
                h2m = sqp.tile([P, NQ], MM, tag="h2m")
                nc.scalar.copy(h2m, h2T[:, cb, :])
                sq2 = sqp.tile([P, NQ], MM, tag="sq2")
                nc.vector.tensor_mul(sq2, h2m, h2m)
                nc.tensor.matmul(ps2_sum, lhsT=ones128, rhs=h2m,
                                 start=(cb == 0), stop=(cb == CB - 1))
                nc.tensor.matmul(ps2_sq, lhsT=ones128, rhs=sq2,
                                 start=(cb == 0), stop=(cb == CB - 1))

            # ---- LN2 tail ---------------------------------------------------
            mu2 = stat.tile([1, 512], F32, tag="mu")
            nc.vector.tensor_scalar_mul(mu2, ps2_sum[0:1, :], 1.0 / DIM)
            musq2 = stat.tile([1, 512], F32, tag="musq")
            nc.vector.tensor_mul(musq2, mu2, mu2)
            var2 = stat.tile([1, 512], F32, tag="var")
            nc.vector.scalar_tensor_tensor(
                out=var2, in0=ps2_sq[0:1, :], scalar=1.0 / DIM, in1=musq2,
                op0=OP.mult, op1=OP.subtract)
            rstd2 = stat.tile([1, 512], F32, tag="rstd")
            nc.scalar.activation(out=rstd2, in_=var2, func=AF.Sqrt,
                                 bias=epst[0:1, :], scale=1.0)
            nc.vector.reciprocal(rstd2, rstd2)
            rstd2_b = stat.tile([P, 512], F32, tag="rstd_b")
            nc.gpsimd.partition_broadcast(rstd2_b, rstd2)
            nmu2 = stat.tile([1, 512], F32, tag="nmu")
            nc.vector.scalar_tensor_tensor(
                out=nmu2, in0=mu2, scalar=-1.0, in1=rstd2,
                op0=OP.mult, op1=OP.mult)
            nmu2_b = stat.tile([P, 512], F32, tag="nmu_b")
            nc.gpsimd.partition_broadcast(nmu2_b, nmu2)
            for cb in range(CB):
                t1 = evac.tile([P, 512], F32, tag="t1")
                nc.vector.tensor_mul(t1, h2T[:, cb, :], rstd2_b)
                nc.vector.tensor_add(t1, t1, nmu2_b)
                nc.vector.tensor_scalar(
                    out=h2T[:, cb, :], in0=t1,
                    scalar1=gam[:, cb:cb + 1], scalar2=bet[:, cb:cb + 1],
                    op0=OP.mult, op1=OP.add)
                nc.sync.dma_start(d_out[cb * P:(cb + 1) * P, :], h2T[:, cb, :])

    nc.compile()
    return nc


# ---- host-side preparation --------------------------------------------------
def _rope_tables():
    inv_freq = 1.0 / (10000.0 ** (np.arange(0, HD, 2, dtype=np.float32) / HD))
    pos = np.arange(L, dtype=np.float32)
    ang = np.einsum("i,j->ij", pos, inv_freq)  # (L, 32)
    return np.cos(ang).astype(np.float32), np.sin(ang).astype(np.float32)


def _prep_in_maps(x, Wq, Wk, Wv, W1, W2, gamma, beta):
    perm = np.concatenate(
        [h * HD + np.concatenate([np.arange(0, HD, 2), np.arange(1, HD, 2)])
         for h in range(HEAD)])
    Wq_p = Wq[:, perm]
    Wk_p = Wk[:, perm]
    cos, sin = _rope_tables()  # (L, 32)

    iidx = np.arange(P) % 32
    sgn = np.where((np.arange(P) // 32) % 2 == 0, -1.0, 1.0).astype(np.float32)

    cosT = cos[:, iidx].T.astype(np.float32)              # (128, L)
    sinT = (sin[:, iidx] * sgn[None, :]).T.astype(np.float32)

    gammaT = gamma.reshape(CB, P).T.astype(np.float32)
    betaT = beta.reshape(CB, P).T.astype(np.float32)

    W1g = (W1 * gamma[:, None]).astype(np.float32)
    w1csum = np.zeros((EB, P, P), np.float32)
    w1csum[:, 0, :] = -(W1g.sum(axis=0, dtype=np.float64)
                        ).astype(np.float32).reshape(EB, P)
    betaW1 = (beta @ W1).astype(np.float32)

    def wlay(w, mblk):  # (DIM_in, M) -> (M//mblk, P, KB, mblk)
        kin = w.shape[0] // P
        return np.ascontiguousarray(
            w.reshape(kin, P, w.shape[1] // mblk, mblk).transpose(2, 1, 0, 3)
        ).astype(NP_MM)

    # triangular mask for the diagonal 128x128 sub-block: key t (partition)
    # masked against query p (col) iff t > p; same for both heads.
    tt = np.arange(P)[:, None]
    pp = np.arange(P)[None, :]
    m2 = np.where(tt > pp, -8000.0, 0.0).astype(np.float32)
    mask2 = np.ascontiguousarray(
        np.broadcast_to(m2[:, None, :], (P, 2, P))).astype(NP_MM)

    # x in channel-major for both batches: [P, CB, B, L]
    xall = np.ascontiguousarray(
        np.stack([x[b].T.reshape(CB, P, L) for b in range(B)], axis=2)
        .transpose(1, 0, 2, 3)).astype(NP_MM)   # (P, CB, B, L)

    com = {
        "xbT2": xall,
        "W1": wlay(W1g, P),
        "W1c": np.ascontiguousarray(w1csum).astype(NP_MM),
        "bW1": np.ascontiguousarray(betaW1.reshape(EB, P).T),
        "W2": np.ascontiguousarray(
            W2.reshape(EB, P, DIM).transpose(1, 0, 2)).astype(NP_MM),
        "cosT": np.ascontiguousarray(cosT),
        "sinT": np.ascontiguousarray(sinT),
        "mask2": mask2,
        "gammaT": np.ascontiguousarray(gammaT),
        "betaT": np.ascontiguousarray(betaT),
    }

    def wslice(w, c):  # per-core head-pair slice -> [P, CB, P]
        cols = w[:, c * P:(c + 1) * P]          # (DIM, 128)
        return np.ascontiguousarray(
            cols.reshape(CB, P, P).transpose(1, 0, 2)).astype(NP_MM)

    in_maps = []
    for core in range(8):
        b, q = core // 4, core % 4
        xq = x[b, q * NQ:(q + 1) * NQ]           # (NQ, D) contiguous quarter
        m = dict(com)
        m["Wq"] = wslice(Wq_p, core)
        m["Wk"] = wslice(Wk_p, core)
        m["Wv"] = wslice(Wv, core)
        m["xqTf"] = np.ascontiguousarray(
            xq.T.reshape(CB, P, NQ).transpose(1, 0, 2)).astype(np.float32)
        in_maps.append(m)
    return in_maps


def _assemble(results):
    out = np.empty((B, L, DIM), dtype=np.float32)
    for core in range(8):
        b, q = core // 4, core % 4
        out[b, q * NQ:(q + 1) * NQ, :] = results[core]["outT"].T
    return out


def _get_program():
    if "nc" not in _CACHE:
        _CACHE["nc"] = _build_program()
    return _CACHE["nc"]


def run(in_maps, trace=False, **kw):
    nc = _get_program()
    return run_bass_kernel_spmd(nc, in_maps, core_ids=list(range(8)),
                                trace=trace, **kw)


def kernel(x, Wq, bq, Wk, bk, Wv, bv, W1, b1, W2, b2, gamma, beta):
    for name, b_ in (("bq", bq), ("bk", bk), ("bv", bv), ("b1", b1), ("b2", b2)):
        if np.abs(np.asarray(b_)).max() != 0.0:
            raise NotImplementedError(f"nonzero bias {name} not supported")
    x = np.asarray(x, dtype=np.float32)
    in_maps = _prep_in_maps(
        x, np.asarray(Wq), np.asarray(Wk), np.asarray(Wv),
        np.asarray(W1), np.asarray(W2), np.asarray(gamma), np.asarray(beta))
    res = run(in_maps, trace=False)
    return _assemble(res.results)
